# revision 7
# baseline (speedup 1.0000x reference)
"""Bass/Tile TRN2 kernel for nn_CRMF_35296041239144.

Social-LSTM-style decoder: mapping MLP on K x B hidden states, then a
12-step LSTM recurrence (hard-sigmoid gates, clipped cell) with a 2-dim
output projection per step.

Sharding: batch 2048 -> 8 cores x 256. Per core rows = K*Bc = 5120.
State is kept transposed [H=128 partitions, rows free] so the recurrent
matmul needs no transposes: gates[gate_unit, row] = W_hh_block.T @ h.
The x-term + bias + hard-sigmoid affine offset are folded into the same
PSUM accumulation via contraction-3 matmuls (x0, x1, 1) placed in
distinct PE row-groups so all four run concurrently.

hard_sigmoid(z) = clip(z/6 + 0.5, 0, 1): the 1/6 scale and +0.5 offset
are pre-folded into the i/f/o weight blocks host-side, so on device the
gate nonlinearity is a single dual-op (max 0, min 1) tensor_scalar.
"""

import numpy as np

import concourse.bass as bass
import concourse.bacc as bacc
import concourse.tile as tile
from concourse import mybir
from concourse.bass_utils import run_bass_kernel_spmd

OBS_LEN, K, B, H, MID, NC_OUT, CIN = 12, 20, 2048, 128, 256, 2, 3
NCORES = 8
BC = B // NCORES            # 256 batch rows per core
ROWS = K * BC               # 5120 rows per core (k-major: r = k*BC + b)
CHUNK = 512
NCH = ROWS // CHUNK         # 10
NTILE = ROWS // 128         # 40 transpose tiles

F32 = mybir.dt.float32
F32R = mybir.dt.float32r
BF16 = mybir.dt.bfloat16
AF = mybir.ActivationFunctionType
OP = mybir.AluOpType

# gate order used on device: [i, f, o, g]; source block order in w_ih/w_hh
# is [i, f, g, o] (reference splits gates into i,f,g,o).
SRC_BLOCK = [0, 1, 3, 2]

# debug ablation switches (timing studies only; results become wrong):
# 'mlp', 'gmm', 'xmm', 'evac', 'dve', 'outproj'
ABLATE = set()


def build_nc(reps: int = 1):
    do_mlp = 'mlp' not in ABLATE
    do_gmm = 'gmm' not in ABLATE
    do_xmm = 'xmm' not in ABLATE
    do_evac = 'evac' not in ABLATE and do_gmm
    do_dve = 'dve' not in ABLATE and do_evac
    do_out = 'outproj' not in ABLATE

    nc = bacc.Bacc("TRN2", target_bir_lowering=False, debug=False)

    ph = nc.dram_tensor("ph", [ROWS, H], F32R, kind="ExternalInput")
    xr = nc.dram_tensor("xr", [3, OBS_LEN, CHUNK], F32R, kind="ExternalInput")
    whh = nc.dram_tensor("whh", [H, 4 * H], F32R, kind="ExternalInput")
    wih = nc.dram_tensor("wih", [128, H], F32R, kind="ExternalInput")
    w0 = nc.dram_tensor("w0", [H, MID], F32R, kind="ExternalInput")
    w1 = nc.dram_tensor("w1", [MID, H], F32R, kind="ExternalInput")
    oww = nc.dram_tensor("oww", [H, NC_OUT], F32R, kind="ExternalInput")
    bpack = nc.dram_tensor("bpack", [128, 4], F32, kind="ExternalInput")
    ident = nc.dram_tensor("ident", [128, 128], F32R, kind="ExternalInput")
    outd = nc.dram_tensor("out", [OBS_LEN, NC_OUT, ROWS], F32,
                          kind="ExternalOutput")

    with tile.TileContext(nc) as tc:
        with tc.tile_pool(name="const", bufs=1) as const, \
             tc.tile_pool(name="state", bufs=1) as state, \
             tc.tile_pool(name="outs", bufs=2) as outs_p:

            whh_sb = const.tile([128, 4 * H], F32R)
            nc.sync.dma_start(out=whh_sb[:], in_=whh[:])
            wih_sb = const.tile([128, H], F32R)
            nc.sync.dma_start(out=wih_sb[:], in_=wih[:])
            w0_sb = const.tile([128, MID], F32R)
            nc.sync.dma_start(out=w0_sb[:], in_=w0[:])
            w1_sb = const.tile([128, 2, H], F32R)
            nc.sync.dma_start(out=w1_sb[:],
                              in_=w1.rearrange("(a p) h -> p a h", p=128))
            oww_sb = const.tile([128, NC_OUT], F32R)
            nc.sync.dma_start(out=oww_sb[:], in_=oww[:])
            bp_sb = const.tile([128, 4], F32)
            nc.sync.dma_start(out=bp_sb[:], in_=bpack[:])
            id_sb = const.tile([128, 128], F32R)
            nc.sync.dma_start(out=id_sb[:], in_=ident[:])
            # x-term moving operand, replicated into 4 partition groups
            xr_sb = const.tile([128, OBS_LEN, CHUNK], F32R)
            for g in range(4):
                nc.sync.dma_start(out=xr_sb[32 * g:32 * g + 3, :, :],
                                  in_=xr[:])

            # per-chunk state tiles so chunk pipelines stay independent
            h_ch = [state.tile([128, CHUNK], F32R, name=f"h{j}",
                               tag=f"h{j}") for j in range(NCH)]
            c_ch = [state.tile([128, CHUNK], BF16, name=f"c{j}",
                               tag=f"c{j}") for j in range(NCH)]

            for _rep in range(reps):
                # ---------- phase 1: transpose ph, mapping MLP ----------
                with tc.tile_pool(name="mlpsb", bufs=1) as mlpsb, \
                     tc.tile_pool(name="h1p", bufs=3) as h1p, \
                     tc.tile_pool(name="pst", bufs=2, space="PSUM") as pst, \
                     tc.tile_pool(name="ps1", bufs=2, space="PSUM") as ps1p, \
                     tc.tile_pool(name="ps0", bufs=2, space="PSUM") as ps0p:

                    for j in range(NCH):
                        nc.vector.memset(c_ch[j][:], 0.0)

                    if do_mlp:
                        ph_nat = mlpsb.tile([128, NTILE, H], F32R,
                                            tag="ph_nat")
                        nc.sync.dma_start(
                            out=ph_nat[:],
                            in_=ph.rearrange("(n p) h -> p n h", p=128))
                        ph_t = mlpsb.tile([128, ROWS], F32R, tag="ph_t")
                        for n in range(NTILE):
                            ptile = pst.tile([128, 128], F32R)
                            nc.tensor.transpose(ptile[:], ph_nat[:, n, :],
                                                id_sb[:])
                            sl = ph_t[:, n * 128:(n + 1) * 128]
                            if n % 2 == 0:
                                nc.vector.tensor_copy(sl, ptile[:])
                            else:
                                nc.scalar.activation(sl, ptile[:], AF.Copy)

                        for j in range(NCH):
                            rs = slice(j * CHUNK, (j + 1) * CHUNK)
                            ps1 = ps1p.tile([128, 2, CHUNK], F32)
                            nc.tensor.matmul(ps1[:, 0, :], w0_sb[:, 0:128],
                                             ph_t[:, rs], start=True,
                                             stop=True)
                            nc.tensor.matmul(ps1[:, 1, :], w0_sb[:, 128:256],
                                             ph_t[:, rs], start=True,
                                             stop=True)
                            h1t = h1p.tile([128, 2, CHUNK], F32R, tag="h1")
                            nc.scalar.activation(h1t[:, 0, :], ps1[:, 0, :],
                                                 AF.Lrelu, bias=bp_sb[:, 0:1],
                                                 alpha=0.01)
                            nc.scalar.activation(h1t[:, 1, :], ps1[:, 1, :],
                                                 AF.Lrelu, bias=bp_sb[:, 1:2],
                                                 alpha=0.01)
                            ps0 = ps0p.tile([128, CHUNK], F32)
                            nc.tensor.matmul(ps0[:], w1_sb[:, 0, :],
                                             h1t[:, 0, :], start=True,
                                             stop=False)
                            nc.tensor.matmul(ps0[:], w1_sb[:, 1, :],
                                             h1t[:, 1, :], start=False,
                                             stop=True)
                            nc.scalar.activation(h_ch[j][:], ps0[:],
                                                 AF.Identity,
                                                 bias=bp_sb[:, 2:3])
                    else:
                        for j in range(NCH):
                            nc.vector.memset(h_ch[j][:], 0.5)

                # ---------- phase 2: LSTM recurrence ----------
                with tc.tile_pool(name="psifo", bufs=2,
                                  space="PSUM") as psifo_p, \
                     tc.tile_pool(name="psg", bufs=1, space="PSUM") as psg_p, \
                     tc.tile_pool(name="pso", bufs=1, space="PSUM") as pso_p, \
                     tc.tile_pool(name="gsb", bufs=3) as gsb_p, \
                     tc.tile_pool(name="tmp", bufs=6) as tmp_p:

                    def out_proj(j, dest):
                        pso = pso_p.tile([NC_OUT, CHUNK], F32, name="pso")
                        nc.tensor.matmul(pso[:], oww_sb[:], h_ch[j][:],
                                         start=True, stop=True)
                        nc.scalar.activation(
                            dest[:, j * CHUNK:(j + 1) * CHUNK], pso[:],
                            AF.Identity, bias=bp_sb[0:NC_OUT, 3:4])

                    outstep = None
                    prev_outstep = None
                    for t in range(OBS_LEN):
                        prev_outstep = outstep
                        outstep = outs_p.tile([NC_OUT, ROWS], F32,
                                              tag="outstep", name="outstep")
                        for j in range(NCH):
                            # output projection of the PREVIOUS step reads h
                            # before this chunk's elementwise overwrites it
                            if t > 0 and do_out:
                                out_proj(j, prev_outstep)

                            if do_gmm:
                                psifo = psifo_p.tile([128, 3, CHUNK], F32,
                                                     name="psifo")
                                psg = psg_p.tile([128, CHUNK], F32,
                                                 name="psg")
                                for gi in range(3):
                                    nc.tensor.matmul(
                                        psifo[:, gi, :],
                                        whh_sb[:, gi * 128:(gi + 1) * 128],
                                        h_ch[j][:], start=True,
                                        stop=not do_xmm)
                                nc.tensor.matmul(psg[:], whh_sb[:, 384:512],
                                                 h_ch[j][:],
                                                 start=True, stop=not do_xmm)
                                if do_xmm:
                                    xop = xr_sb[:, t, :]
                                    for gi in range(3):
                                        nc.tensor.matmul(
                                            psifo[:, gi, :],
                                            wih_sb[32 * gi:32 * gi + 3, :],
                                            xop[32 * gi:32 * gi + 3, :],
                                            start=False, stop=True,
                                            tile_position=(32 * gi, 0))
                                    nc.tensor.matmul(psg[:], wih_sb[96:99, :],
                                                     xop[96:99, :],
                                                     start=False, stop=True,
                                                     tile_position=(96, 0))

                            if do_evac:
                                # evac i,f,o to SBUF (ACT), clamp there (4x)
                                ifo = gsb_p.tile([128, 3, CHUNK], BF16,
                                                 tag="ifo", name="ifo")
                                nc.scalar.activation(ifo[:], psifo[:],
                                                     AF.Relu)
                            if do_dve:
                                # g: clamp straight out of PSUM
                                gt = gsb_p.tile([128, CHUNK], BF16, tag="g",
                                                name="gt")
                                nc.vector.tensor_scalar(
                                    out=gt[:], in0=psg[:], scalar1=1.0,
                                    scalar2=-1.0, op0=OP.min, op1=OP.max)
                                # c' = f*c + i*g ; cc = clip(c') ; h = o*cc
                                t1 = tmp_p.tile([128, CHUNK], BF16, tag="t1",
                                                name="t1")
                                nc.vector.scalar_tensor_tensor(
                                    out=t1[:], in0=ifo[:, 1, :], scalar=1.0,
                                    in1=c_ch[j][:], op0=OP.min, op1=OP.mult)
                                t2 = tmp_p.tile([128, CHUNK], BF16, tag="t2",
                                                name="t2")
                                nc.vector.scalar_tensor_tensor(
                                    out=t2[:], in0=ifo[:, 0, :], scalar=1.0,
                                    in1=gt[:], op0=OP.min, op1=OP.mult)
                                nc.vector.tensor_tensor(
                                    out=c_ch[j][:], in0=t1[:], in1=t2[:],
                                    op=OP.add)
                                cc = tmp_p.tile([128, CHUNK], BF16, tag="cc",
                                                name="cc")
                                nc.vector.tensor_scalar(
                                    out=cc[:], in0=c_ch[j][:], scalar1=1.0,
                                    scalar2=-1.0, op0=OP.min, op1=OP.max)
                                nc.vector.scalar_tensor_tensor(
                                    out=h_ch[j][:], in0=ifo[:, 2, :],
                                    scalar=1.0, in1=cc[:], op0=OP.min,
                                    op1=OP.mult)
                        if t > 0 and do_out:
                            nc.sync.dma_start(out=outd[t - 1],
                                              in_=prev_outstep[:])

                    if do_out:
                        for j in range(NCH):
                            out_proj(j, outstep)
                        nc.sync.dma_start(out=outd[OBS_LEN - 1],
                                          in_=outstep[:])

    nc.finalize()
    return nc


def prep_inputs(obs_traj_rel, pred_lstm_hidden, map_w0, map_b0, map_w1,
                map_b1, w_ih, w_hh, b_ih, b_hh, out_w, out_b):
    """Host-side prep -> list of per-core input dicts."""
    f32 = np.float32
    bias = (np.asarray(b_ih, f32) + np.asarray(b_hh, f32))
    w_hh = np.asarray(w_hh, f32)
    w_ih = np.asarray(w_ih, f32)

    whh_stat = np.empty((H, 4 * H), f32)
    wih_stat = np.zeros((128, H), f32)
    for gi in range(4):
        sb = SRC_BLOCK[gi]
        s = (1.0 / 6.0) if gi < 3 else 1.0
        off = 0.5 if gi < 3 else 0.0
        whh_stat[:, gi * 128:(gi + 1) * 128] = \
            w_hh[sb * 128:(sb + 1) * 128].T * s
        wih_stat[32 * gi + 0:32 * gi + 2, :] = \
            w_ih[sb * 128:(sb + 1) * 128, :].T * s
        wih_stat[32 * gi + 2, :] = bias[sb * 128:(sb + 1) * 128] * s + off

    bpack = np.zeros((128, 4), f32)
    bpack[:, 0] = np.asarray(map_b0, f32)[0:128]
    bpack[:, 1] = np.asarray(map_b0, f32)[128:256]
    bpack[:, 2] = np.asarray(map_b1, f32)
    bpack[0:NC_OUT, 3] = np.asarray(out_b, f32)

    obs = np.asarray(obs_traj_rel, f32)
    xs = np.concatenate([obs[0:1], obs[:-1]], axis=0)[:, :, 0:2]  # [T,B,2]
    ph_full = np.asarray(pred_lstm_hidden, f32)

    common = dict(
        whh=whh_stat, wih=wih_stat,
        w0=np.ascontiguousarray(np.asarray(map_w0, f32)),
        w1=np.ascontiguousarray(np.asarray(map_w1, f32)),
        oww=np.ascontiguousarray(np.asarray(out_w, f32)),
        bpack=bpack, ident=np.eye(128, dtype=f32),
    )
    in_maps = []
    for c in range(NCORES):
        bs = slice(c * BC, (c + 1) * BC)
        ph_core = np.ascontiguousarray(
            ph_full[:, bs, :].reshape(ROWS, H))
        x_core = xs[:, bs, :]                       # [T, BC, 2]
        xr_core = np.empty((3, OBS_LEN, CHUNK), f32)
        for t in range(OBS_LEN):
            for rep in range(CHUNK // BC):
                xr_core[0, t, rep * BC:(rep + 1) * BC] = x_core[t, :, 0]
                xr_core[1, t, rep * BC:(rep + 1) * BC] = x_core[t, :, 1]
        xr_core[2] = 1.0
        in_maps.append(dict(ph=ph_core, xr=xr_core, **common))
    return in_maps


def assemble_output(results):
    """Per-core [T, 2, ROWS] (k-major rows) -> full [T, K, B, 2]."""
    out = np.empty((OBS_LEN, K, B, NC_OUT), np.float32)
    for c, res in enumerate(results):
        o = res["out"].reshape(OBS_LEN, NC_OUT, K, BC)
        out[:, :, c * BC:(c + 1) * BC, :] = o.transpose(0, 2, 3, 1)
    return out


def kernel(**inputs):
    nc = build_nc(reps=1)
    in_maps = prep_inputs(**inputs)
    res = run_bass_kernel_spmd(nc, in_maps, core_ids=list(range(NCORES)))
    return assemble_output(res.results)


if __name__ == "__main__":
    import reference as R
    inputs = {k: np.asarray(v) for k, v in R.setup_inputs().items()}
    got = kernel(**inputs)
    import jax.numpy as jnp
    ref = np.asarray(
        R.reference(**{k: jnp.asarray(v) for k, v in inputs.items()}))
    err = np.abs(got - ref).max()
    rel = err / np.abs(ref).max()
    print(f"absmax={err:.4e} rel={rel:.4e}")


# revision 8
# speedup vs baseline: 10.7563x; 10.7563x over previous
"""Bass/Tile TRN2 kernel for nn_CRMF_35296041239144.

Social-LSTM-style decoder: mapping MLP on K x B hidden states, then a
12-step LSTM recurrence (hard-sigmoid gates, clipped cell) with a 2-dim
output projection per step.

Sharding: batch 2048 -> 8 cores x 256. Per core rows = K*Bc = 5120.
State is kept transposed [H=128 partitions, rows free] so the recurrent
matmul needs no transposes: gates[gate_unit, row] = W_hh_block.T @ h.
The x-term + bias + hard-sigmoid affine offset are folded into the same
PSUM accumulation via contraction-3 matmuls (x0, x1, 1) placed in
distinct PE row-groups so all four run concurrently.

hard_sigmoid(z) = clip(z/6 + 0.5, 0, 1): the 1/6 scale and +0.5 offset
are pre-folded into the i/f/o weight blocks host-side, so on device the
gate nonlinearity is a single dual-op (max 0, min 1) tensor_scalar.
"""

import numpy as np
from contextlib import nullcontext

import concourse.bass as bass
import concourse.bacc as bacc
import concourse.tile as tile
from concourse import mybir
from concourse.bass_utils import run_bass_kernel_spmd

OBS_LEN, K, B, H, MID, NC_OUT, CIN = 12, 20, 2048, 128, 256, 2, 3
NCORES = 8
BC = B // NCORES            # 256 batch rows per core
ROWS = K * BC               # 5120 rows per core (k-major: r = k*BC + b)
CHUNK = 512
NCH = ROWS // CHUNK         # 10
NTILE = ROWS // 128         # 40 transpose tiles

F32 = mybir.dt.float32
F32R = mybir.dt.float32r
BF16 = mybir.dt.bfloat16
AF = mybir.ActivationFunctionType
OP = mybir.AluOpType

# gate order used on device: [i, f, o, g]; source block order in w_ih/w_hh
# is [i, f, g, o] (reference splits gates into i,f,g,o).
SRC_BLOCK = [0, 1, 3, 2]

# debug ablation switches (timing studies only; results become wrong):
# 'mlp', 'gmm', 'xmm', 'evac', 'dve', 'outproj'
ABLATE = set()


def build_nc(reps: int = 1):
    do_mlp = 'mlp' not in ABLATE
    do_gmm = 'gmm' not in ABLATE
    do_xmm = 'xmm' not in ABLATE
    do_evac = 'evac' not in ABLATE and do_gmm
    do_dve = 'dve' not in ABLATE and do_evac
    do_out = 'outproj' not in ABLATE

    nc = bacc.Bacc("TRN2", target_bir_lowering=False, debug=False)

    ph = nc.dram_tensor("ph", [ROWS, H], F32R, kind="ExternalInput")
    xr = nc.dram_tensor("xr", [3, OBS_LEN, CHUNK], F32R, kind="ExternalInput")
    whh = nc.dram_tensor("whh", [H, 4 * H], F32R, kind="ExternalInput")
    wih = nc.dram_tensor("wih", [128, H], F32R, kind="ExternalInput")
    w0 = nc.dram_tensor("w0", [H, MID], F32R, kind="ExternalInput")
    w1 = nc.dram_tensor("w1", [MID, H], F32R, kind="ExternalInput")
    oww = nc.dram_tensor("oww", [H, NC_OUT], F32R, kind="ExternalInput")
    bpack = nc.dram_tensor("bpack", [128, 4], F32, kind="ExternalInput")
    ident = nc.dram_tensor("ident", [128, 128], F32R, kind="ExternalInput")
    outd = nc.dram_tensor("out", [OBS_LEN, NC_OUT, ROWS], F32,
                          kind="ExternalOutput")

    with tile.TileContext(nc) as tc:
        with tc.tile_pool(name="const", bufs=1) as const, \
             tc.tile_pool(name="state", bufs=1) as state, \
             tc.tile_pool(name="outs", bufs=2) as outs_p:

            whh_sb = const.tile([128, 4 * H], F32R)
            nc.sync.dma_start(out=whh_sb[:], in_=whh[:])
            wih_sb = const.tile([128, H], F32R)
            nc.sync.dma_start(out=wih_sb[:], in_=wih[:])
            w0_sb = const.tile([128, MID], F32R)
            nc.sync.dma_start(out=w0_sb[:], in_=w0[:])
            w1_sb = const.tile([128, 2, H], F32R)
            nc.sync.dma_start(out=w1_sb[:],
                              in_=w1.rearrange("(a p) h -> p a h", p=128))
            oww_sb = const.tile([128, NC_OUT], F32R)
            nc.sync.dma_start(out=oww_sb[:], in_=oww[:])
            bp_sb = const.tile([128, 4], F32)
            nc.sync.dma_start(out=bp_sb[:], in_=bpack[:])
            id_sb = const.tile([128, 128], F32R)
            nc.sync.dma_start(out=id_sb[:], in_=ident[:])
            # x-term moving operand, replicated into 4 partition groups
            xr_sb = const.tile([128, OBS_LEN, CHUNK], F32R)
            for g in range(4):
                nc.sync.dma_start(out=xr_sb[32 * g:32 * g + 3, :, :],
                                  in_=xr[:])

            # per-chunk state tiles so chunk pipelines stay independent
            h_ch = [state.tile([128, CHUNK], F32R, name=f"h{j}",
                               tag=f"h{j}") for j in range(NCH)]
            c_ch = [state.tile([128, CHUNK], BF16, name=f"c{j}",
                               tag=f"c{j}") for j in range(NCH)]

            with (tc.For_i(0, reps, 1) if reps > 1 else nullcontext()):
                # ---------- phase 1: transpose ph, mapping MLP ----------
                with tc.tile_pool(name="mlpsb", bufs=1) as mlpsb, \
                     tc.tile_pool(name="h1p", bufs=3) as h1p, \
                     tc.tile_pool(name="pst", bufs=2, space="PSUM") as pst, \
                     tc.tile_pool(name="ps1", bufs=2, space="PSUM") as ps1p, \
                     tc.tile_pool(name="ps0", bufs=2, space="PSUM") as ps0p:

                    for j in range(NCH):
                        nc.vector.memset(c_ch[j][:], 0.0)

                    if do_mlp:
                        ph_nat = mlpsb.tile([128, NTILE, H], F32R,
                                            tag="ph_nat")
                        nc.sync.dma_start(
                            out=ph_nat[:],
                            in_=ph.rearrange("(n p) h -> p n h", p=128))
                        ph_t = mlpsb.tile([128, ROWS], F32R, tag="ph_t")
                        for n in range(NTILE):
                            ptile = pst.tile([128, 128], F32R)
                            nc.tensor.transpose(ptile[:], ph_nat[:, n, :],
                                                id_sb[:])
                            sl = ph_t[:, n * 128:(n + 1) * 128]
                            if n % 2 == 0:
                                nc.vector.tensor_copy(sl, ptile[:])
                            else:
                                nc.scalar.activation(sl, ptile[:], AF.Copy)

                        for j in range(NCH):
                            rs = slice(j * CHUNK, (j + 1) * CHUNK)
                            ps1 = ps1p.tile([128, 2, CHUNK], F32)
                            nc.tensor.matmul(ps1[:, 0, :], w0_sb[:, 0:128],
                                             ph_t[:, rs], start=True,
                                             stop=True)
                            nc.tensor.matmul(ps1[:, 1, :], w0_sb[:, 128:256],
                                             ph_t[:, rs], start=True,
                                             stop=True)
                            h1t = h1p.tile([128, 2, CHUNK], F32R, tag="h1")
                            nc.scalar.activation(h1t[:, 0, :], ps1[:, 0, :],
                                                 AF.Lrelu, bias=bp_sb[:, 0:1],
                                                 alpha=0.01)
                            nc.scalar.activation(h1t[:, 1, :], ps1[:, 1, :],
                                                 AF.Lrelu, bias=bp_sb[:, 1:2],
                                                 alpha=0.01)
                            ps0 = ps0p.tile([128, CHUNK], F32)
                            nc.tensor.matmul(ps0[:], w1_sb[:, 0, :],
                                             h1t[:, 0, :], start=True,
                                             stop=False)
                            nc.tensor.matmul(ps0[:], w1_sb[:, 1, :],
                                             h1t[:, 1, :], start=False,
                                             stop=True)
                            nc.scalar.activation(h_ch[j][:], ps0[:],
                                                 AF.Identity,
                                                 bias=bp_sb[:, 2:3])
                    else:
                        for j in range(NCH):
                            nc.vector.memset(h_ch[j][:], 0.5)

                # ---------- phase 2: LSTM recurrence ----------
                with tc.tile_pool(name="psifo", bufs=2,
                                  space="PSUM") as psifo_p, \
                     tc.tile_pool(name="psg", bufs=1, space="PSUM") as psg_p, \
                     tc.tile_pool(name="pso", bufs=1, space="PSUM") as pso_p, \
                     tc.tile_pool(name="gsb", bufs=3) as gsb_p, \
                     tc.tile_pool(name="tmp", bufs=6) as tmp_p:

                    def out_proj(j, dest):
                        pso = pso_p.tile([NC_OUT, CHUNK], F32, name="pso")
                        nc.tensor.matmul(pso[:], oww_sb[:], h_ch[j][:],
                                         start=True, stop=True)
                        nc.scalar.activation(
                            dest[:, j * CHUNK:(j + 1) * CHUNK], pso[:],
                            AF.Identity, bias=bp_sb[0:NC_OUT, 3:4])

                    outstep = None
                    prev_outstep = None
                    for t in range(OBS_LEN):
                        prev_outstep = outstep
                        outstep = outs_p.tile([NC_OUT, ROWS], F32,
                                              tag="outstep", name="outstep")
                        for j in range(NCH):
                            # output projection of the PREVIOUS step reads h
                            # before this chunk's elementwise overwrites it
                            if t > 0 and do_out:
                                out_proj(j, prev_outstep)

                            if do_gmm:
                                psifo = psifo_p.tile([128, 3, CHUNK], F32,
                                                     name="psifo")
                                psg = psg_p.tile([128, CHUNK], F32,
                                                 name="psg")
                                for gi in range(3):
                                    nc.tensor.matmul(
                                        psifo[:, gi, :],
                                        whh_sb[:, gi * 128:(gi + 1) * 128],
                                        h_ch[j][:], start=True,
                                        stop=not do_xmm)
                                nc.tensor.matmul(psg[:], whh_sb[:, 384:512],
                                                 h_ch[j][:],
                                                 start=True, stop=not do_xmm)
                                if do_xmm:
                                    xop = xr_sb[:, t, :]
                                    for gi in range(3):
                                        nc.tensor.matmul(
                                            psifo[:, gi, :],
                                            wih_sb[32 * gi:32 * gi + 3, :],
                                            xop[32 * gi:32 * gi + 3, :],
                                            start=False, stop=True,
                                            tile_position=(32 * gi, 0))
                                    nc.tensor.matmul(psg[:], wih_sb[96:99, :],
                                                     xop[96:99, :],
                                                     start=False, stop=True,
                                                     tile_position=(96, 0))

                            if do_evac:
                                # evac i,f,o to SBUF (ACT), clamp there (4x)
                                ifo = gsb_p.tile([128, 3, CHUNK], BF16,
                                                 tag="ifo", name="ifo")
                                nc.scalar.activation(ifo[:], psifo[:],
                                                     AF.Relu)
                            if do_dve:
                                # g: clamp straight out of PSUM
                                gt = gsb_p.tile([128, CHUNK], BF16, tag="g",
                                                name="gt")
                                nc.vector.tensor_scalar(
                                    out=gt[:], in0=psg[:], scalar1=1.0,
                                    scalar2=-1.0, op0=OP.min, op1=OP.max)
                                # c' = f*c + i*g ; cc = clip(c') ; h = o*cc
                                t1 = tmp_p.tile([128, CHUNK], BF16, tag="t1",
                                                name="t1")
                                nc.vector.scalar_tensor_tensor(
                                    out=t1[:], in0=ifo[:, 1, :], scalar=1.0,
                                    in1=c_ch[j][:], op0=OP.min, op1=OP.mult)
                                t2 = tmp_p.tile([128, CHUNK], BF16, tag="t2",
                                                name="t2")
                                nc.vector.scalar_tensor_tensor(
                                    out=t2[:], in0=ifo[:, 0, :], scalar=1.0,
                                    in1=gt[:], op0=OP.min, op1=OP.mult)
                                nc.vector.tensor_tensor(
                                    out=c_ch[j][:], in0=t1[:], in1=t2[:],
                                    op=OP.add)
                                cc = tmp_p.tile([128, CHUNK], BF16, tag="cc",
                                                name="cc")
                                nc.vector.tensor_scalar(
                                    out=cc[:], in0=c_ch[j][:], scalar1=1.0,
                                    scalar2=-1.0, op0=OP.min, op1=OP.max)
                                nc.vector.scalar_tensor_tensor(
                                    out=h_ch[j][:], in0=ifo[:, 2, :],
                                    scalar=1.0, in1=cc[:], op0=OP.min,
                                    op1=OP.mult)
                        if t > 0 and do_out:
                            nc.sync.dma_start(out=outd[t - 1],
                                              in_=prev_outstep[:])

                    if do_out:
                        for j in range(NCH):
                            out_proj(j, outstep)
                        nc.sync.dma_start(out=outd[OBS_LEN - 1],
                                          in_=outstep[:])

    nc.finalize()
    return nc


def prep_inputs(obs_traj_rel, pred_lstm_hidden, map_w0, map_b0, map_w1,
                map_b1, w_ih, w_hh, b_ih, b_hh, out_w, out_b):
    """Host-side prep -> list of per-core input dicts."""
    f32 = np.float32
    bias = (np.asarray(b_ih, f32) + np.asarray(b_hh, f32))
    w_hh = np.asarray(w_hh, f32)
    w_ih = np.asarray(w_ih, f32)

    whh_stat = np.empty((H, 4 * H), f32)
    wih_stat = np.zeros((128, H), f32)
    for gi in range(4):
        sb = SRC_BLOCK[gi]
        s = (1.0 / 6.0) if gi < 3 else 1.0
        off = 0.5 if gi < 3 else 0.0
        whh_stat[:, gi * 128:(gi + 1) * 128] = \
            w_hh[sb * 128:(sb + 1) * 128].T * s
        wih_stat[32 * gi + 0:32 * gi + 2, :] = \
            w_ih[sb * 128:(sb + 1) * 128, :].T * s
        wih_stat[32 * gi + 2, :] = bias[sb * 128:(sb + 1) * 128] * s + off

    bpack = np.zeros((128, 4), f32)
    bpack[:, 0] = np.asarray(map_b0, f32)[0:128]
    bpack[:, 1] = np.asarray(map_b0, f32)[128:256]
    bpack[:, 2] = np.asarray(map_b1, f32)
    bpack[0:NC_OUT, 3] = np.asarray(out_b, f32)

    obs = np.asarray(obs_traj_rel, f32)
    xs = np.concatenate([obs[0:1], obs[:-1]], axis=0)[:, :, 0:2]  # [T,B,2]
    ph_full = np.asarray(pred_lstm_hidden, f32)

    common = dict(
        whh=whh_stat, wih=wih_stat,
        w0=np.ascontiguousarray(np.asarray(map_w0, f32)),
        w1=np.ascontiguousarray(np.asarray(map_w1, f32)),
        oww=np.ascontiguousarray(np.asarray(out_w, f32)),
        bpack=bpack, ident=np.eye(128, dtype=f32),
    )
    in_maps = []
    for c in range(NCORES):
        bs = slice(c * BC, (c + 1) * BC)
        ph_core = np.ascontiguousarray(
            ph_full[:, bs, :].reshape(ROWS, H))
        x_core = xs[:, bs, :]                       # [T, BC, 2]
        xr_core = np.empty((3, OBS_LEN, CHUNK), f32)
        for t in range(OBS_LEN):
            for rep in range(CHUNK // BC):
                xr_core[0, t, rep * BC:(rep + 1) * BC] = x_core[t, :, 0]
                xr_core[1, t, rep * BC:(rep + 1) * BC] = x_core[t, :, 1]
        xr_core[2] = 1.0
        in_maps.append(dict(ph=ph_core, xr=xr_core, **common))
    return in_maps


def assemble_output(results):
    """Per-core [T, 2, ROWS] (k-major rows) -> full [T, K, B, 2]."""
    out = np.empty((OBS_LEN, K, B, NC_OUT), np.float32)
    for c, res in enumerate(results):
        o = res["out"].reshape(OBS_LEN, NC_OUT, K, BC)
        out[:, :, c * BC:(c + 1) * BC, :] = o.transpose(0, 2, 3, 1)
    return out


def kernel(**inputs):
    nc = build_nc(reps=1)
    in_maps = prep_inputs(**inputs)
    res = run_bass_kernel_spmd(nc, in_maps, core_ids=list(range(NCORES)))
    return assemble_output(res.results)


if __name__ == "__main__":
    import reference as R
    inputs = {k: np.asarray(v) for k, v in R.setup_inputs().items()}
    got = kernel(**inputs)
    import jax.numpy as jnp
    ref = np.asarray(
        R.reference(**{k: jnp.asarray(v) for k, v in inputs.items()}))
    err = np.abs(got - ref).max()
    rel = err / np.abs(ref).max()
    print(f"absmax={err:.4e} rel={rel:.4e}")
